# revision 40
# baseline (speedup 1.0000x reference)
"""Multi-head attention (12 heads, head_dim 64, RoPE, seq 1024) on 8 trn2 cores.

Sharding: pure data-parallel over the 16 (batch, row) units -> 2 per core.
No collectives. Each core runs the full per-unit attention.

Final design (measured 358.0us on HW vs 432.7us baseline, rel err
1.2e-3 vs the 2e-2 gate):
  - fp16 everywhere on the PE/DVE; x host-transposed to [unit, p, dj, s]
    so each SBUF partition's 12KB loads with one DMA descriptor.
  - startup DMAs split across BOTH hwdge queues (SP + ACT; each queue
    is FIFO) in dj-SLICES interleaved by first consumption (dma deps
    are slice-granular, and the first qk matmul chain consumes (xT dj,
    wq dj) in order): first matmul at ~11us instead of ~32us.
  - softmax-sum (ones) matmuls run BEFORE the PV matmuls in each
    block, so the 1.8us InstReciprocal overlaps the PV streams and the
    next block's PV start (psum-bank WAR on the normalize) never
    stalls the PE queue.
  - hp-outer attention blocks per unit; block j's scores are emitted
    two blocks before block j's PV (defer-2), so the PE queue head
    never blocks on the scalar engine's exp stream.
  - STAGGERED units (lockstep measured worse by ~15us): unit 1 runs
    ~10 blocks behind and its projections drip 1 step/round as the
    mid-kernel PE filler; out-projections fill the drain.
  - scores pairs run concurrently via tile_position row split, PV and
    softmax-sum (ones) pairs via column split; LDWEIGHTS hides under
    the N=256 streams (~110ns per pair-slot).
  - engine floors per core: ACT 192 exps ~214us, DVE ~227us (84us of
    which is InstReciprocal at 6.86ns/elem - no faster path exists:
    custom-DVE ops and TT-divide fail codegen, ACT Reciprocal is
    blocked and table-thrashes), PE ~290us streams+overheads. PE binds.

  biases: bq/bk applied in-kernel; bv/bo folded on the host:
  out += bv @ Wo + bo (exact: sum(probs)=1).
  mask: all-ones fast path; any zero -> exact numpy fallback.

  This walrus build encodes at most ONE semaphore wait per instruction;
  _legalize_waits() hoists excess waits into preceding same-engine NoOps.
"""
import numpy as np

H = 768
NH = 12
HD = 64
S = 1024
P = 128
DT = H // P          # 6 din/dout tiles
ST = S // P          # 8 seq tiles
BR = 2               # (b,r) units per core
NCORES = 8
QC = 256             # q-chunk
NQC = S // QC        # 4
NB = DT * NQC        # 24 attention blocks per unit (hp-outer)
ROPE_BASE = 10000.0

_CACHE = {}


def _rope_tables():
    inv = 1.0 / (ROPE_BASE ** (np.arange(0, HD, 2, dtype=np.float64) / HD))  # [32]
    t = np.arange(S, dtype=np.float64)
    f = np.outer(inv, t)                      # [32, S]
    cos2 = np.zeros((P, S), dtype=np.float16)
    sins = np.zeros((P, S), dtype=np.float16)
    c = np.cos(f).astype(np.float16)
    s = np.sin(f).astype(np.float16)
    for p in range(P):
        cos2[p] = c[p % 32]
        sins[p] = -s[p % 32] if (p % 64) < 32 else s[p % 32]
    return cos2, sins


def _legalize_waits(nc):
    """This walrus encodes at most one sync wait per instruction: hoist
    excess waits onto preceding same-engine NoOps."""
    import concourse.mybir as mybir

    n = 0
    for f in nc.m.functions:
        for blk in f.blocks:
            new = []
            for inst in blk.instructions:
                si = inst.sync_info
                waits = list(si.on_wait) if si and si.on_wait else []
                if len(waits) > 1:
                    for i, w in enumerate(waits[:-1]):
                        nop = mybir.InstNoOp(
                            name=f"{inst.name}-wn{i}", ins=[], outs=[],
                            sync_info=mybir.SyncInfo(on_wait=[w], on_update=[]))
                        nop.engine = inst.engine
                        new.append(nop)
                        n += 1
                    inst.sync_info = mybir.SyncInfo(
                        on_wait=[waits[-1]],
                        on_update=list(si.on_update) if si.on_update else [])
                new.append(inst)
            blk.instructions = new
    return n


def _build():
    import concourse.bass as bass
    import concourse.mybir as mybir
    import concourse.tile as tile
    from collections import deque

    F32 = mybir.dt.float32
    F16 = mybir.dt.float16
    Exp = mybir.ActivationFunctionType.Exp
    MUL = mybir.AluOpType.mult
    ADD = mybir.AluOpType.add
    DIV = mybir.AluOpType.divide

    nc = bass.Bass()
    # xsT host layout [unit, p, dj, s]: one contiguous 12KB run per
    # partition -> a single dma_start with one descriptor per partition.
    xsT = nc.dram_tensor("xsT", [BR, P, DT, S], F16, kind="ExternalInput")
    wq = nc.dram_tensor("wq", [P, DT, H], F16, kind="ExternalInput")
    wk = nc.dram_tensor("wk", [P, DT, H], F16, kind="ExternalInput")
    wv = nc.dram_tensor("wv", [P, DT, H], F16, kind="ExternalInput")
    wo = nc.dram_tensor("wo", [P, DT, H], F16, kind="ExternalInput")
    bqk = nc.dram_tensor("bqk", [P, 2, DT], F32, kind="ExternalInput")
    cossin = nc.dram_tensor("cossin", [P, 2, S], F16, kind="ExternalInput")
    out = nc.dram_tensor("out", [BR, S, H], F32, kind="ExternalOutput")

    with tile.TileContext(nc) as tc:
        with tc.tile_pool(name="const", bufs=1) as cpool, \
             tc.tile_pool(name="wpool", bufs=1) as wpool, \
             tc.tile_pool(name="xT", bufs=1) as xT_pool, \
             tc.tile_pool(name="qk", bufs=1) as qk_pool, \
             tc.tile_pool(name="rope", bufs=2) as rope_pool, \
             tc.tile_pool(name="vp", bufs=1) as v_pool, \
             tc.tile_pool(name="probs", bufs=4) as probs_pool, \
             tc.tile_pool(name="den", bufs=2) as den_pool, \
             tc.tile_pool(name="at", bufs=1) as at_pool, \
             tc.tile_pool(name="ot", bufs=2) as ot_pool, \
             tc.tile_pool(name="sc", bufs=2, space="PSUM") as sc_pool, \
             tc.tile_pool(name="pv", bufs=1, space="PSUM") as pv_pool, \
             tc.tile_pool(name="pj", bufs=2, space="PSUM") as ppj_pool:

            # ---- startup: DMAs split over BOTH hwdge queues (SP +
            # Activation), ordered first-consumed-first. A single queue
            # is FIFO: v6 had bqk (needed by the first bias add) queued
            # behind 3.8MB, pushing the first exp to 46us.
            ones64 = cpool.tile([P, HD], F16, tag="ones")
            nc.gpsimd.memset(ones64[:], 1.0)

            # PE warm-up: the HAM clock gate keeps the array at 1.2GHz
            # until ~3.4us of sustained activity. The PE would otherwise
            # idle through the ~11us input-DMA window and run the first
            # ~24 projection matmuls cold (634ns vs 379ns at N=512);
            # burn the wait on dummy matmuls over zeroed scratch so the
            # real chain starts at full clock.
            wsc = cpool.tile([P, 576], F16, tag="warmsc")
            nc.vector.memset(wsc[:], 0.0)
            wps = ppj_pool.tile([P, 512], F32, tag="pj")
            for _ in range(18):
                nc.tensor.matmul(wps[0:64, 0:512], wsc[:, 0:64],
                                 wsc[:, 64:576], start=True, stop=True)

            xt_sb = {}

            def emit_xT(br, eng):
                t = xT_pool.tile([P, DT, S], F16, tag=f"xT_{br}",
                                 name=f"xT_{br}")
                # halves land as two transfers so the queue interleaves
                eng.dma_start(t[:, 0:3], xsT[br, :, 0:3])
                eng.dma_start(t[:, 3:6], xsT[br, :, 3:6])
                xt_sb[br] = t

            w_sb = {}

            def emit_w(name, w, eng):
                w_sb[name] = wpool.tile([P, DT, H], F16, tag=f"w{name}",
                                        name=f"w{name}")
                eng.dma_start(w_sb[name][:], w[:])

            bqk_sb = cpool.tile([P, 2, DT], F32, tag="bqk")
            nc.scalar.dma_start(bqk_sb[:], bqk[:])
            # The first qk matmul chain consumes (xT0 dj, wq dj) slices
            # IN ORDER, and dma_start deps are slice-granular: split the
            # two tensors into dj-slices interleaved across both queues
            # so the dj0 slices land in ~11us and the rest stream ahead
            # of the accumulation chain (v9 gated 25us on the FULL wq
            # queued behind xT0's second half).
            t0 = xT_pool.tile([P, DT, S], F16, tag="xT_0", name="xT_0")
            xt_sb[0] = t0
            w_sb["q"] = wpool.tile([P, DT, H], F16, tag="wq", name="wq")
            nc.sync.dma_start(t0[:, 0:2], xsT[0, :, 0:2])
            nc.scalar.dma_start(w_sb["q"][:, 0:2], wq[:, 0:2])
            nc.scalar.dma_start(w_sb["q"][:, 2:6], wq[:, 2:6])
            nc.sync.dma_start(t0[:, 2:4], xsT[0, :, 2:4])
            nc.scalar.dma_start(t0[:, 4:6], xsT[0, :, 4:6])
            emit_w("k", wk, nc.sync)
            cs_sb = cpool.tile([P, 2, S], F16, tag="cossin")
            nc.scalar.dma_start(cs_sb[:], cossin[:])
            cos_sb = cs_sb[:, 0]
            sin_sb = cs_sb[:, 1]
            emit_w("v", wv, nc.sync)
            emit_xT(1, nc.scalar)
            emit_w("o", wo, nc.sync)

            # ---- per-unit state ----
            state = {br: {"v": [None] * ST,
                          "qkT": {"q": [None] * DT, "k": [None] * DT},
                          "at": at_pool.tile([P, NQC, DT, QC], F16,
                                             tag=f"at_{br}", name=f"at_{br}"),
                          "pr": [None] * NB}
                     for br in range(BR)}

            def v_step(br, st):
                xT = xt_sb[br]
                vt = v_pool.tile([P, H], F16, tag=f"v{st}_{br}",
                                 name=f"v{st}_{br}")
                state[br]["v"][st] = vt
                for nb in range(2):
                    c0 = nb * 384
                    pp = ppj_pool.tile([P, 512], F32, tag="pj")
                    for dj in range(DT):
                        nc.tensor.matmul(
                            pp[:, 0:384],
                            xT[:, dj, st * P:(st + 1) * P],
                            w_sb["v"][:, dj, c0:c0 + 384],
                            start=(dj == 0), stop=(dj == DT - 1))
                    nc.vector.tensor_copy(vt[:, c0:c0 + 384], pp[:, 0:384])

            def qk_step(br, name, qi, tt):
                xT = xt_sb[br]
                # rotating tag: qk tile tt is dead once stretch tt's
                # scores are done, so slots alternate (WAR tracked).
                dst = qk_pool.tile([P, S], F16, tag=f"{name}{tt % 2}_{br}",
                                   name=f"{name}T{tt}_{br}")
                state[br]["qkT"][name][tt] = dst
                for half in range(2):
                    pp = ppj_pool.tile([P, 512], F32, tag="pj")
                    for dj in range(DT):
                        nc.tensor.matmul(
                            pp[:, 0:512],
                            w_sb[name][:, dj, tt * P:(tt + 1) * P],
                            xT[:, dj, half * 512:(half + 1) * 512],
                            start=(dj == 0), stop=(dj == DT - 1))
                    nc.vector.tensor_scalar_add(
                        dst[:, half * 512:(half + 1) * 512],
                        pp[:, 0:512], bqk_sb[:, qi, tt:tt + 1])
                # RoPE: dst = dst*cos + swap(dst)*sins
                sw = rope_pool.tile([P, S], F16, tag="ropesw")
                for hh2 in range(2):
                    b0 = hh2 * 64
                    nc.sync.dma_start(sw[b0:b0 + 32, :],
                                      dst[b0 + 32:b0 + 64, :])
                    nc.sync.dma_start(sw[b0 + 32:b0 + 64, :],
                                      dst[b0:b0 + 32, :])
                nc.vector.tensor_tensor(sw[:], sw[:], sin_sb, MUL)
                nc.vector.tensor_tensor(dst[:], dst[:], cos_sb, MUL)
                nc.vector.tensor_tensor(dst[:], dst[:], sw[:], ADD)

            def sc_step(br, i):
                hp, qc = divmod(i, NQC)
                q0 = qc * QC
                qkT = state[br]["qkT"]
                pr = probs_pool.tile([P, 4, 1024], F16, tag=f"pr_{br}",
                                     name=f"pr{i}_{br}")
                state[br]["pr"][i] = pr
                for g in range(4):
                    sc_ps = sc_pool.tile([P, 1024], F32, tag="sc")
                    for i2 in range(2):
                        kt = 2 * g + i2
                        for hh, base in ((0, 0), (1, 64)):
                            nc.tensor.matmul(
                                sc_ps[:, hh * 512 + i2 * QC:
                                      hh * 512 + (i2 + 1) * QC],
                                qkT["k"][hp][base:base + 64,
                                             kt * P:(kt + 1) * P],
                                qkT["q"][hp][base:base + 64, q0:q0 + QC],
                                start=True, stop=True,
                                tile_position=(base, 0))
                    nc.scalar.activation(pr[:, g, :], sc_ps[:], Exp,
                                         scale=0.125)

            def pv_step(br, i):
                hp, qc = divmod(i, NQC)
                pr = state[br]["pr"][i]
                v_sb = state[br]["v"]
                pva = pv_pool.tile([P, 512], F32, tag="pva")
                pvs = pv_pool.tile([P, 512], F32, tag="pvs")
                # ones (softmax sums) FIRST: the slow reciprocal then
                # overlaps the PV streams instead of running after them,
                # pulling the normalize chain ~1us earlier (the next
                # block's PV start waits on it via the psum-bank WAR).
                for kt in range(ST):
                    nc.tensor.matmul(
                        pvs[0:64, 0:QC], ones64[:],
                        pr[:, kt // 2, (kt % 2) * QC:(kt % 2) * QC + QC],
                        start=(kt == 0), stop=(kt == ST - 1),
                        tile_position=(0, 0))
                    nc.tensor.matmul(
                        pvs[64:128, 0:QC], ones64[:],
                        pr[:, kt // 2, 512 + (kt % 2) * QC:
                           512 + (kt % 2) * QC + QC],
                        start=(kt == 0), stop=(kt == ST - 1),
                        tile_position=(0, 64), skip_group_check=True)
                rec = den_pool.tile([P, QC], F32, tag="den")
                nc.vector.reciprocal(rec[:], pvs[:, 0:QC])
                for kt in range(ST):
                    nc.tensor.matmul(
                        pva[0:64, 0:QC],
                        v_sb[kt][:, (2 * hp) * HD:(2 * hp + 1) * HD],
                        pr[:, kt // 2, (kt % 2) * QC:(kt % 2) * QC + QC],
                        start=(kt == 0), stop=(kt == ST - 1),
                        tile_position=(0, 0))
                    nc.tensor.matmul(
                        pva[64:128, 0:QC],
                        v_sb[kt][:, (2 * hp + 1) * HD:(2 * hp + 2) * HD],
                        pr[:, kt // 2, 512 + (kt % 2) * QC:
                           512 + (kt % 2) * QC + QC],
                        start=(kt == 0), stop=(kt == ST - 1),
                        tile_position=(0, 64), skip_group_check=True)
                at = state[br]["at"]
                nc.vector.tensor_tensor(at[:, qc, hp, :], pva[:, 0:QC],
                                        rec[:], MUL)

            def op_step(br, qc, sc2, drain=False):
                at = state[br]["at"]
                ot = ot_pool.tile([P, H], F32, tag="ot")
                for nb in range(2):
                    c0 = nb * 384
                    po = ppj_pool.tile([P, 512], F32, tag="pj")
                    for dj in range(DT):
                        nc.tensor.matmul(
                            po[:, 0:384],
                            at[:, qc, dj, sc2 * P:(sc2 + 1) * P],
                            w_sb["o"][:, dj, c0:c0 + 384],
                            start=(dj == 0), stop=(dj == DT - 1))
                    r0 = qc * QC + sc2 * P
                    if drain:
                        # exps are done by the drain phase -- use the
                        # idle scalar engine for the psum evacuation and
                        # ship each half as soon as it lands
                        nc.scalar.copy(ot[:, c0:c0 + 384], po[:, 0:384])
                        nc.sync.dma_start(out[br, r0:r0 + P, c0:c0 + 384],
                                          ot[:, c0:c0 + 384])
                    else:
                        nc.vector.tensor_copy(ot[:, c0:c0 + 384],
                                              po[:, 0:384])
                if not drain:
                    r0 = qc * QC + sc2 * P
                    nc.sync.dma_start(out[br, r0:r0 + P, :], ot[:])

            # ---------------- schedule ----------------
            # Staggered units (lockstep measured WORSE: unit-1's
            # projections are the only mid-kernel PE filler; spreading
            # them 1/round through unit-0's attention keeps the PE dense
            # where the exp-waits open bubbles). Unit 0 bootstraps with
            # q0/k0 + hp0 scores so the exp stream lights early.
            qk_step(0, "q", 0, 0)
            qk_step(0, "k", 1, 0)
            sc_step(0, 0)
            sc_step(0, 1)
            v_step(0, 0)
            v_step(0, 1)
            v_step(0, 2)
            v_step(0, 3)
            sc_step(0, 2)
            v_step(0, 4)
            v_step(0, 5)
            v_step(0, 6)
            v_step(0, 7)
            pv_step(0, 0)
            qk_step(0, "q", 0, 1)
            qk_step(0, "k", 1, 1)
            sc_step(0, 3)
            pv_step(0, 1)

            sc_i = {0: 4, 1: 0}
            pv_i = {0: 2, 1: 0}
            qk_pairs = {0: 2, 1: 0}
            v_done = {0: True, 1: False}
            F0 = deque((tt, n, qi) for tt in range(2, DT)
                       for n, qi in (("q", 0), ("k", 1)))
            F1 = deque([("v", st, None) for st in range(ST)] +
                       [("qk", n, tt) for tt in range(DT)
                        for n in ("q", "k")])
            OP = deque()
            f1_v = 0

            def emit_f1():
                nonlocal f1_v
                if not F1:
                    return False
                kind, a, b = F1.popleft()
                if kind == "v":
                    v_step(1, a)
                    f1_v += 1
                    if f1_v == ST:
                        v_done[1] = True
                else:
                    qk_step(1, a, 0 if a == "q" else 1, b)
                    if a == "k":
                        qk_pairs[1] += 1
                return True

            def maybe_sc(u):
                i = sc_i[u]
                if i >= NB or i - pv_i[u] >= 4:
                    return False
                if i // NQC >= qk_pairs[u]:
                    return False
                sc_step(u, i)
                sc_i[u] += 1
                return True

            def maybe_pv(u, defer=2):
                # defer-2: pv(j) goes out only after sc(j+2), so the exp
                # batch it waits on is ~2 ACT-rounds old -> the PE queue
                # head never blocks on the scalar engine.
                j = pv_i[u]
                if j >= sc_i[u] - defer and sc_i[u] < NB:
                    return False
                if j >= min(sc_i[u], NB) or not v_done[u]:
                    return False
                pv_step(u, j)
                pv_i[u] += 1
                hp, qc = divmod(j, NQC)
                if hp == DT - 1:
                    OP.append((u, qc, 0))
                    OP.append((u, qc, 1))
                return True

            while (pv_i[0] < NB or pv_i[1] < NB or F0 or F1 or OP):
                progress = False
                for u in (0, 1):
                    # just-in-time qk pair for unit0's next stretch
                    while F0 and sc_i[0] >= 4 * qk_pairs[0] - 2:
                        tt, n, qi = F0.popleft()
                        qk_step(0, n, qi, tt)
                        if n == "k":
                            qk_pairs[0] += 1
                        progress = True
                    progress |= maybe_sc(u)
                    # filler inside the leg: it sits BETWEEN pv(u0) and
                    # pv(u1) in the PE queue, covering the pva-bank WAR
                    # latency (normalize of the other unit's block).
                    if u == 0 and F1:
                        progress |= emit_f1()
                    elif OP:
                        ou, oqc, osc2 = OP.popleft()
                        op_step(ou, oqc, osc2)
                        progress = True
                    progress |= maybe_pv(u)
                if not progress:
                    # drain stragglers: alternate remaining pvs and
                    # out-projections so the PE tail stays dense
                    for u in (0, 1):
                        progress |= maybe_pv(u, defer=0)
                        if OP:
                            ou, oqc, osc2 = OP.popleft()
                            op_step(ou, oqc, osc2,
                                    drain=(pv_i[0] == NB and pv_i[1] == NB))
                            progress = True
                    assert progress, "schedule deadlock"

    _legalize_waits(nc)
    return nc


def _get_nc():
    if "nc" not in _CACHE:
        _CACHE["nc"] = _build()
    return _CACHE["nc"]


def _numpy_reference(x, Wq, bq, Wk, bk, Wv, bv, Wo, bo, mask):
    b, r, s, d = x.shape
    inv = 1.0 / (ROPE_BASE ** (np.arange(0, HD, 2, dtype=np.float32) / HD))
    t = np.arange(s, dtype=np.float32)
    f = np.outer(t, inv)
    emb = np.concatenate([f, f], axis=-1)
    cos, sin = np.cos(emb), np.sin(emb)

    def proj(W, bvec):
        y = x @ W + bvec
        return y.reshape(b, r, s, NH, HD).transpose(0, 1, 3, 2, 4)

    def rot(z):
        z1, z2 = z[..., :HD // 2], z[..., HD // 2:]
        return np.concatenate([-z2, z1], axis=-1)

    q = proj(Wq, bq)
    k = proj(Wk, bk)
    v = proj(Wv, bv)
    q = q * cos + rot(q) * sin
    k = k * cos + rot(k) * sin
    scores = np.einsum("brhqd,brhkd->brhqk", q, k) / np.sqrt(np.float32(HD))
    scores = np.where(mask == 0, -np.inf, scores)
    m = scores.max(axis=-1, keepdims=True)
    e = np.exp(scores - m)
    probs = e / e.sum(axis=-1, keepdims=True)
    o = np.einsum("brhqk,brhkd->brhqd", probs, v)
    o = o.transpose(0, 1, 3, 2, 4).reshape(b, r, s, d)
    return (o @ Wo + bo).astype(np.float32)


def _run(inputs, trace=False):
    from concourse.bass_utils import run_bass_kernel_spmd

    x = np.asarray(inputs["x"], dtype=np.float32)
    Wq = np.asarray(inputs["Wq"], dtype=np.float32)
    Wk = np.asarray(inputs["Wk"], dtype=np.float32)
    Wv = np.asarray(inputs["Wv"], dtype=np.float32)
    Wo = np.asarray(inputs["Wo"], dtype=np.float32)
    bq = np.asarray(inputs["bq"], dtype=np.float32)
    bk = np.asarray(inputs["bk"], dtype=np.float32)
    bv = np.asarray(inputs["bv"], dtype=np.float32)
    bo = np.asarray(inputs["bo"], dtype=np.float32)

    # host-side prep: x to [unit, p, dj, seq] fp16 (contiguous per
    # SBUF partition -> single-descriptor DMA)
    xf = x.reshape(NCORES * BR, S, DT, P).transpose(0, 3, 2, 1)
    xf = np.ascontiguousarray(xf).astype(np.float16)
    # weights [p, t, o] fp16: row (t*128 + p) of W -> [p, t, :]
    def wprep(W):
        return np.ascontiguousarray(
            W.reshape(DT, P, H).transpose(1, 0, 2)).astype(np.float16)
    # biases [p, 2, t]
    bqk_h = np.ascontiguousarray(np.stack(
        [bq.reshape(DT, P).T, bk.reshape(DT, P).T], axis=1)).astype(np.float32)

    cos2, sins = _rope_tables()
    cossin = np.ascontiguousarray(np.stack([cos2, sins], axis=1))
    nc = _get_nc()
    in_maps = []
    wq_h, wk_h, wv_h, wo_h = wprep(Wq), wprep(Wk), wprep(Wv), wprep(Wo)
    for c in range(NCORES):
        in_maps.append(dict(
            xsT=np.ascontiguousarray(xf[c * BR:(c + 1) * BR]),
            wq=wq_h, wk=wk_h, wv=wv_h, wo=wo_h, bqk=bqk_h,
            cossin=cossin))
    kw = {}
    if trace:
        import os
        td = "/tmp/trn_trace"
        os.makedirs(td, exist_ok=True)
        kw["tmpdir"] = td
    res = run_bass_kernel_spmd(nc, in_maps, core_ids=list(range(NCORES)),
                               trace=trace, **kw)
    outs = np.concatenate([r["out"] for r in res.results], axis=0)
    out = outs.reshape(2, NCORES * BR // 2, S, H)
    out = out + (bv @ Wo + bo)
    return out.astype(np.float32), res


def kernel(**inputs):
    mask = np.asarray(inputs["mask"])
    if not np.all(mask != 0):
        return _numpy_reference(
            x=np.asarray(inputs["x"], np.float32),
            Wq=np.asarray(inputs["Wq"], np.float32),
            bq=np.asarray(inputs["bq"], np.float32),
            Wk=np.asarray(inputs["Wk"], np.float32),
            bk=np.asarray(inputs["bk"], np.float32),
            Wv=np.asarray(inputs["Wv"], np.float32),
            bv=np.asarray(inputs["bv"], np.float32),
            Wo=np.asarray(inputs["Wo"], np.float32),
            bo=np.asarray(inputs["bo"], np.float32),
            mask=mask)
    out, _ = _run(inputs, trace=False)
    return out


# revision 43
# speedup vs baseline: 1.0066x; 1.0066x over previous
"""Multi-head attention (12 heads, head_dim 64, RoPE, seq 1024) on 8 trn2 cores.

Sharding: pure data-parallel over the 16 (batch, row) units -> 2 per core.
No collectives. Each core runs the full per-unit attention.

Final design (measured 358.0us on HW vs 432.7us baseline, rel err
1.2e-3 vs the 2e-2 gate):
  - fp16 everywhere on the PE/DVE; x host-transposed to [unit, p, dj, s]
    so each SBUF partition's 12KB loads with one DMA descriptor.
  - startup DMAs split across BOTH hwdge queues (SP + ACT; each queue
    is FIFO) in dj-SLICES interleaved by first consumption (dma deps
    are slice-granular, and the first qk matmul chain consumes (xT dj,
    wq dj) in order): first matmul at ~11us instead of ~32us.
  - softmax-sum (ones) matmuls run BEFORE the PV matmuls in each
    block, so the 1.8us InstReciprocal overlaps the PV streams and the
    next block's PV start (psum-bank WAR on the normalize) never
    stalls the PE queue.
  - hp-outer attention blocks per unit; block j's scores are emitted
    two blocks before block j's PV (defer-2), so the PE queue head
    never blocks on the scalar engine's exp stream.
  - STAGGERED units (lockstep measured worse by ~15us): unit 1 runs
    ~10 blocks behind and its projections drip 1 step/round as the
    mid-kernel PE filler; out-projections fill the drain.
  - scores pairs run concurrently via tile_position row split, PV and
    softmax-sum (ones) pairs via column split; LDWEIGHTS hides under
    the N=256 streams (~110ns per pair-slot).
  - engine floors per core: ACT 192 exps ~214us, DVE ~227us (84us of
    which is InstReciprocal at 6.86ns/elem - no faster path exists:
    custom-DVE ops and TT-divide fail codegen, ACT Reciprocal is
    blocked and table-thrashes), PE ~290us streams+overheads. PE binds.

  biases: bq/bk applied in-kernel; bv/bo folded on the host:
  out += bv @ Wo + bo (exact: sum(probs)=1).
  mask: all-ones fast path; any zero -> exact numpy fallback.

  This walrus build encodes at most ONE semaphore wait per instruction;
  _legalize_waits() hoists excess waits into preceding same-engine NoOps.
"""
import numpy as np

H = 768
NH = 12
HD = 64
S = 1024
P = 128
DT = H // P          # 6 din/dout tiles
ST = S // P          # 8 seq tiles
BR = 2               # (b,r) units per core
NCORES = 8
QC = 256             # q-chunk
NQC = S // QC        # 4
NB = DT * NQC        # 24 attention blocks per unit (hp-outer)
ROPE_BASE = 10000.0

_CACHE = {}


def _rope_tables():
    inv = 1.0 / (ROPE_BASE ** (np.arange(0, HD, 2, dtype=np.float64) / HD))  # [32]
    t = np.arange(S, dtype=np.float64)
    f = np.outer(inv, t)                      # [32, S]
    cos2 = np.zeros((P, S), dtype=np.float16)
    sins = np.zeros((P, S), dtype=np.float16)
    c = np.cos(f).astype(np.float16)
    s = np.sin(f).astype(np.float16)
    for p in range(P):
        cos2[p] = c[p % 32]
        sins[p] = -s[p % 32] if (p % 64) < 32 else s[p % 32]
    return cos2, sins


def _legalize_waits(nc):
    """This walrus encodes at most one sync wait per instruction: hoist
    excess waits onto preceding same-engine NoOps."""
    import concourse.mybir as mybir

    n = 0
    for f in nc.m.functions:
        for blk in f.blocks:
            new = []
            for inst in blk.instructions:
                si = inst.sync_info
                waits = list(si.on_wait) if si and si.on_wait else []
                if len(waits) > 1:
                    for i, w in enumerate(waits[:-1]):
                        nop = mybir.InstNoOp(
                            name=f"{inst.name}-wn{i}", ins=[], outs=[],
                            sync_info=mybir.SyncInfo(on_wait=[w], on_update=[]))
                        nop.engine = inst.engine
                        new.append(nop)
                        n += 1
                    inst.sync_info = mybir.SyncInfo(
                        on_wait=[waits[-1]],
                        on_update=list(si.on_update) if si.on_update else [])
                new.append(inst)
            blk.instructions = new
    return n


def _build():
    import concourse.bass as bass
    import concourse.mybir as mybir
    import concourse.tile as tile
    from collections import deque

    F32 = mybir.dt.float32
    F16 = mybir.dt.float16
    Exp = mybir.ActivationFunctionType.Exp
    MUL = mybir.AluOpType.mult
    ADD = mybir.AluOpType.add
    DIV = mybir.AluOpType.divide

    nc = bass.Bass()
    # xsT host layout [unit, p, dj, s]: one contiguous 12KB run per
    # partition -> a single dma_start with one descriptor per partition.
    xsT = nc.dram_tensor("xsT", [BR, P, DT, S], F16, kind="ExternalInput")
    wq = nc.dram_tensor("wq", [P, DT, H], F16, kind="ExternalInput")
    wk = nc.dram_tensor("wk", [P, DT, H], F16, kind="ExternalInput")
    wv = nc.dram_tensor("wv", [P, DT, H], F16, kind="ExternalInput")
    wo = nc.dram_tensor("wo", [P, DT, H], F16, kind="ExternalInput")
    bqk = nc.dram_tensor("bqk", [P, 2, DT], F32, kind="ExternalInput")
    cossin = nc.dram_tensor("cossin", [P, 2, S], F16, kind="ExternalInput")
    out = nc.dram_tensor("out", [BR, S, H], F32, kind="ExternalOutput")

    with tile.TileContext(nc) as tc:
        with tc.tile_pool(name="const", bufs=1) as cpool, \
             tc.tile_pool(name="wpool", bufs=1) as wpool, \
             tc.tile_pool(name="xT", bufs=1) as xT_pool, \
             tc.tile_pool(name="qk", bufs=1) as qk_pool, \
             tc.tile_pool(name="rope", bufs=2) as rope_pool, \
             tc.tile_pool(name="vp", bufs=1) as v_pool, \
             tc.tile_pool(name="probs", bufs=4) as probs_pool, \
             tc.tile_pool(name="den", bufs=2) as den_pool, \
             tc.tile_pool(name="at", bufs=1) as at_pool, \
             tc.tile_pool(name="ot", bufs=3) as ot_pool, \
             tc.tile_pool(name="sc", bufs=2, space="PSUM") as sc_pool, \
             tc.tile_pool(name="pv", bufs=1, space="PSUM") as pv_pool, \
             tc.tile_pool(name="pj", bufs=2, space="PSUM") as ppj_pool:

            # ---- startup: DMAs split over BOTH hwdge queues (SP +
            # Activation), ordered first-consumed-first. A single queue
            # is FIFO: v6 had bqk (needed by the first bias add) queued
            # behind 3.8MB, pushing the first exp to 46us.
            ones64 = cpool.tile([P, HD], F16, tag="ones")
            nc.gpsimd.memset(ones64[:], 1.0)

            xt_sb = {}

            def emit_xT(br, eng):
                t = xT_pool.tile([P, DT, S], F16, tag=f"xT_{br}",
                                 name=f"xT_{br}")
                # halves land as two transfers so the queue interleaves
                eng.dma_start(t[:, 0:3], xsT[br, :, 0:3])
                eng.dma_start(t[:, 3:6], xsT[br, :, 3:6])
                xt_sb[br] = t

            w_sb = {}

            def emit_w(name, w, eng):
                w_sb[name] = wpool.tile([P, DT, H], F16, tag=f"w{name}",
                                        name=f"w{name}")
                eng.dma_start(w_sb[name][:], w[:])

            bqk_sb = cpool.tile([P, 2, DT], F32, tag="bqk")
            nc.scalar.dma_start(bqk_sb[:], bqk[:])
            # The first qk matmul chain consumes (xT0 dj, wq dj) slices
            # IN ORDER, and dma_start deps are slice-granular: split the
            # two tensors into dj-slices interleaved across both queues
            # so the dj0 slices land in ~11us and the rest stream ahead
            # of the accumulation chain (v9 gated 25us on the FULL wq
            # queued behind xT0's second half).
            t0 = xT_pool.tile([P, DT, S], F16, tag="xT_0", name="xT_0")
            xt_sb[0] = t0
            w_sb["q"] = wpool.tile([P, DT, H], F16, tag="wq", name="wq")
            nc.sync.dma_start(t0[:, 0:2], xsT[0, :, 0:2])
            nc.scalar.dma_start(w_sb["q"][:, 0:2], wq[:, 0:2])
            nc.scalar.dma_start(w_sb["q"][:, 2:6], wq[:, 2:6])
            nc.sync.dma_start(t0[:, 2:4], xsT[0, :, 2:4])
            nc.scalar.dma_start(t0[:, 4:6], xsT[0, :, 4:6])
            emit_w("k", wk, nc.sync)
            cs_sb = cpool.tile([P, 2, S], F16, tag="cossin")
            nc.scalar.dma_start(cs_sb[:], cossin[:])
            cos_sb = cs_sb[:, 0]
            sin_sb = cs_sb[:, 1]
            emit_w("v", wv, nc.sync)
            emit_xT(1, nc.scalar)
            emit_w("o", wo, nc.sync)

            # ---- per-unit state ----
            state = {br: {"v": [None] * ST,
                          "qkT": {"q": [None] * DT, "k": [None] * DT},
                          "at": at_pool.tile([P, NQC, DT, QC], F16,
                                             tag=f"at_{br}", name=f"at_{br}"),
                          "pr": [None] * NB}
                     for br in range(BR)}

            def v_step(br, st):
                xT = xt_sb[br]
                vt = v_pool.tile([P, H], F16, tag=f"v{st}_{br}",
                                 name=f"v{st}_{br}")
                state[br]["v"][st] = vt
                for nb in range(2):
                    c0 = nb * 384
                    pp = ppj_pool.tile([P, 512], F32, tag="pj")
                    for dj in range(DT):
                        nc.tensor.matmul(
                            pp[:, 0:384],
                            xT[:, dj, st * P:(st + 1) * P],
                            w_sb["v"][:, dj, c0:c0 + 384],
                            start=(dj == 0), stop=(dj == DT - 1))
                    nc.vector.tensor_copy(vt[:, c0:c0 + 384], pp[:, 0:384])

            def qk_step(br, name, qi, tt):
                xT = xt_sb[br]
                # rotating tag: qk tile tt is dead once stretch tt's
                # scores are done, so slots alternate (WAR tracked).
                dst = qk_pool.tile([P, S], F16, tag=f"{name}{tt % 2}_{br}",
                                   name=f"{name}T{tt}_{br}")
                state[br]["qkT"][name][tt] = dst
                for half in range(2):
                    pp = ppj_pool.tile([P, 512], F32, tag="pj")
                    for dj in range(DT):
                        nc.tensor.matmul(
                            pp[:, 0:512],
                            w_sb[name][:, dj, tt * P:(tt + 1) * P],
                            xT[:, dj, half * 512:(half + 1) * 512],
                            start=(dj == 0), stop=(dj == DT - 1))
                    nc.vector.tensor_scalar_add(
                        dst[:, half * 512:(half + 1) * 512],
                        pp[:, 0:512], bqk_sb[:, qi, tt:tt + 1])
                # RoPE: dst = dst*cos + swap(dst)*sins
                sw = rope_pool.tile([P, S], F16, tag="ropesw")
                for hh2 in range(2):
                    b0 = hh2 * 64
                    nc.sync.dma_start(sw[b0:b0 + 32, :],
                                      dst[b0 + 32:b0 + 64, :])
                    nc.sync.dma_start(sw[b0 + 32:b0 + 64, :],
                                      dst[b0:b0 + 32, :])
                nc.vector.tensor_tensor(sw[:], sw[:], sin_sb, MUL)
                nc.vector.tensor_tensor(dst[:], dst[:], cos_sb, MUL)
                nc.vector.tensor_tensor(dst[:], dst[:], sw[:], ADD)

            def sc_step(br, i):
                hp, qc = divmod(i, NQC)
                q0 = qc * QC
                qkT = state[br]["qkT"]
                pr = probs_pool.tile([P, 4, 1024], F16, tag=f"pr_{br}",
                                     name=f"pr{i}_{br}")
                state[br]["pr"][i] = pr
                for g in range(4):
                    sc_ps = sc_pool.tile([P, 1024], F32, tag="sc")
                    for i2 in range(2):
                        kt = 2 * g + i2
                        for hh, base in ((0, 0), (1, 64)):
                            nc.tensor.matmul(
                                sc_ps[:, hh * 512 + i2 * QC:
                                      hh * 512 + (i2 + 1) * QC],
                                qkT["k"][hp][base:base + 64,
                                             kt * P:(kt + 1) * P],
                                qkT["q"][hp][base:base + 64, q0:q0 + QC],
                                start=True, stop=True,
                                tile_position=(base, 0))
                    nc.scalar.activation(pr[:, g, :], sc_ps[:], Exp,
                                         scale=0.125)

            def pv_step(br, i):
                hp, qc = divmod(i, NQC)
                pr = state[br]["pr"][i]
                v_sb = state[br]["v"]
                pva = pv_pool.tile([P, 512], F32, tag="pva")
                pvs = pv_pool.tile([P, 512], F32, tag="pvs")
                # ones (softmax sums) FIRST: the slow reciprocal then
                # overlaps the PV streams instead of running after them,
                # pulling the normalize chain ~1us earlier (the next
                # block's PV start waits on it via the psum-bank WAR).
                for kt in range(ST):
                    nc.tensor.matmul(
                        pvs[0:64, 0:QC], ones64[:],
                        pr[:, kt // 2, (kt % 2) * QC:(kt % 2) * QC + QC],
                        start=(kt == 0), stop=(kt == ST - 1),
                        tile_position=(0, 0))
                    nc.tensor.matmul(
                        pvs[64:128, 0:QC], ones64[:],
                        pr[:, kt // 2, 512 + (kt % 2) * QC:
                           512 + (kt % 2) * QC + QC],
                        start=(kt == 0), stop=(kt == ST - 1),
                        tile_position=(0, 64), skip_group_check=True)
                rec = den_pool.tile([P, QC], F32, tag="den")
                nc.vector.reciprocal(rec[:], pvs[:, 0:QC])
                for kt in range(ST):
                    nc.tensor.matmul(
                        pva[0:64, 0:QC],
                        v_sb[kt][:, (2 * hp) * HD:(2 * hp + 1) * HD],
                        pr[:, kt // 2, (kt % 2) * QC:(kt % 2) * QC + QC],
                        start=(kt == 0), stop=(kt == ST - 1),
                        tile_position=(0, 0))
                    nc.tensor.matmul(
                        pva[64:128, 0:QC],
                        v_sb[kt][:, (2 * hp + 1) * HD:(2 * hp + 2) * HD],
                        pr[:, kt // 2, 512 + (kt % 2) * QC:
                           512 + (kt % 2) * QC + QC],
                        start=(kt == 0), stop=(kt == ST - 1),
                        tile_position=(0, 64), skip_group_check=True)
                at = state[br]["at"]
                nc.vector.tensor_tensor(at[:, qc, hp, :], pva[:, 0:QC],
                                        rec[:], MUL)

            def op_step(br, qc, sc2, drain=False):
                at = state[br]["at"]
                ot = ot_pool.tile([P, H], F32, tag="ot")
                for nb in range(2):
                    c0 = nb * 384
                    po = ppj_pool.tile([P, 512], F32, tag="pj")
                    for dj in range(DT):
                        nc.tensor.matmul(
                            po[:, 0:384],
                            at[:, qc, dj, sc2 * P:(sc2 + 1) * P],
                            w_sb["o"][:, dj, c0:c0 + 384],
                            start=(dj == 0), stop=(dj == DT - 1))
                    r0 = qc * QC + sc2 * P
                    if drain:
                        # exps are done by the drain phase -- use the
                        # idle scalar engine for the psum evacuation and
                        # ship each half as soon as it lands
                        nc.scalar.copy(ot[:, c0:c0 + 384], po[:, 0:384])
                        # scalar hwdge queue: parallel to sync's
                        # in-flight out-DMA backlog at the drain
                        nc.scalar.dma_start(out[br, r0:r0 + P, c0:c0 + 384],
                                            ot[:, c0:c0 + 384])
                    else:
                        nc.vector.tensor_copy(ot[:, c0:c0 + 384],
                                              po[:, 0:384])
                if not drain:
                    r0 = qc * QC + sc2 * P
                    nc.sync.dma_start(out[br, r0:r0 + P, :], ot[:])

            # ---------------- schedule ----------------
            # Staggered units (lockstep measured WORSE: unit-1's
            # projections are the only mid-kernel PE filler; spreading
            # them 1/round through unit-0's attention keeps the PE dense
            # where the exp-waits open bubbles). Unit 0 bootstraps with
            # q0/k0 + hp0 scores so the exp stream lights early.
            qk_step(0, "q", 0, 0)
            qk_step(0, "k", 1, 0)
            sc_step(0, 0)
            sc_step(0, 1)
            v_step(0, 0)
            v_step(0, 1)
            v_step(0, 2)
            v_step(0, 3)
            sc_step(0, 2)
            v_step(0, 4)
            v_step(0, 5)
            v_step(0, 6)
            v_step(0, 7)
            pv_step(0, 0)
            qk_step(0, "q", 0, 1)
            qk_step(0, "k", 1, 1)
            sc_step(0, 3)
            pv_step(0, 1)

            sc_i = {0: 4, 1: 0}
            pv_i = {0: 2, 1: 0}
            qk_pairs = {0: 2, 1: 0}
            v_done = {0: True, 1: False}
            F0 = deque((tt, n, qi) for tt in range(2, DT)
                       for n, qi in (("q", 0), ("k", 1)))
            F1 = deque([("v", st, None) for st in range(ST)] +
                       [("qk", n, tt) for tt in range(DT)
                        for n in ("q", "k")])
            OP = deque()
            f1_v = 0

            def emit_f1():
                nonlocal f1_v
                if not F1:
                    return False
                kind, a, b = F1.popleft()
                if kind == "v":
                    v_step(1, a)
                    f1_v += 1
                    if f1_v == ST:
                        v_done[1] = True
                else:
                    qk_step(1, a, 0 if a == "q" else 1, b)
                    if a == "k":
                        qk_pairs[1] += 1
                return True

            def maybe_sc(u):
                i = sc_i[u]
                if i >= NB or i - pv_i[u] >= 4:
                    return False
                if i // NQC >= qk_pairs[u]:
                    return False
                sc_step(u, i)
                sc_i[u] += 1
                return True

            def maybe_pv(u, defer=2):
                # defer-2: pv(j) goes out only after sc(j+2), so the exp
                # batch it waits on is ~2 ACT-rounds old -> the PE queue
                # head never blocks on the scalar engine.
                j = pv_i[u]
                if j >= sc_i[u] - defer and sc_i[u] < NB:
                    return False
                if j >= min(sc_i[u], NB) or not v_done[u]:
                    return False
                pv_step(u, j)
                pv_i[u] += 1
                hp, qc = divmod(j, NQC)
                if hp == DT - 1:
                    OP.append((u, qc, 0))
                    OP.append((u, qc, 1))
                return True

            while (pv_i[0] < NB or pv_i[1] < NB or F0 or F1 or OP):
                progress = False
                for u in (0, 1):
                    # just-in-time qk pair for unit0's next stretch
                    while F0 and sc_i[0] >= 4 * qk_pairs[0] - 2:
                        tt, n, qi = F0.popleft()
                        qk_step(0, n, qi, tt)
                        if n == "k":
                            qk_pairs[0] += 1
                        progress = True
                    progress |= maybe_sc(u)
                    # filler inside the leg: it sits BETWEEN pv(u0) and
                    # pv(u1) in the PE queue, covering the pva-bank WAR
                    # latency (normalize of the other unit's block).
                    if u == 0 and F1:
                        progress |= emit_f1()
                    elif OP:
                        ou, oqc, osc2 = OP.popleft()
                        op_step(ou, oqc, osc2)
                        progress = True
                    progress |= maybe_pv(u)
                if not progress:
                    # drain stragglers: alternate remaining pvs and
                    # out-projections so the PE tail stays dense
                    for u in (0, 1):
                        progress |= maybe_pv(u, defer=0)
                        if OP:
                            ou, oqc, osc2 = OP.popleft()
                            op_step(ou, oqc, osc2,
                                    drain=(pv_i[0] == NB and pv_i[1] == NB))
                            progress = True
                    assert progress, "schedule deadlock"

    _legalize_waits(nc)
    return nc


def _get_nc():
    if "nc" not in _CACHE:
        _CACHE["nc"] = _build()
    return _CACHE["nc"]


def _numpy_reference(x, Wq, bq, Wk, bk, Wv, bv, Wo, bo, mask):
    b, r, s, d = x.shape
    inv = 1.0 / (ROPE_BASE ** (np.arange(0, HD, 2, dtype=np.float32) / HD))
    t = np.arange(s, dtype=np.float32)
    f = np.outer(t, inv)
    emb = np.concatenate([f, f], axis=-1)
    cos, sin = np.cos(emb), np.sin(emb)

    def proj(W, bvec):
        y = x @ W + bvec
        return y.reshape(b, r, s, NH, HD).transpose(0, 1, 3, 2, 4)

    def rot(z):
        z1, z2 = z[..., :HD // 2], z[..., HD // 2:]
        return np.concatenate([-z2, z1], axis=-1)

    q = proj(Wq, bq)
    k = proj(Wk, bk)
    v = proj(Wv, bv)
    q = q * cos + rot(q) * sin
    k = k * cos + rot(k) * sin
    scores = np.einsum("brhqd,brhkd->brhqk", q, k) / np.sqrt(np.float32(HD))
    scores = np.where(mask == 0, -np.inf, scores)
    m = scores.max(axis=-1, keepdims=True)
    e = np.exp(scores - m)
    probs = e / e.sum(axis=-1, keepdims=True)
    o = np.einsum("brhqk,brhkd->brhqd", probs, v)
    o = o.transpose(0, 1, 3, 2, 4).reshape(b, r, s, d)
    return (o @ Wo + bo).astype(np.float32)


def _run(inputs, trace=False):
    from concourse.bass_utils import run_bass_kernel_spmd

    x = np.asarray(inputs["x"], dtype=np.float32)
    Wq = np.asarray(inputs["Wq"], dtype=np.float32)
    Wk = np.asarray(inputs["Wk"], dtype=np.float32)
    Wv = np.asarray(inputs["Wv"], dtype=np.float32)
    Wo = np.asarray(inputs["Wo"], dtype=np.float32)
    bq = np.asarray(inputs["bq"], dtype=np.float32)
    bk = np.asarray(inputs["bk"], dtype=np.float32)
    bv = np.asarray(inputs["bv"], dtype=np.float32)
    bo = np.asarray(inputs["bo"], dtype=np.float32)

    # host-side prep: x to [unit, p, dj, seq] fp16 (contiguous per
    # SBUF partition -> single-descriptor DMA)
    xf = x.reshape(NCORES * BR, S, DT, P).transpose(0, 3, 2, 1)
    xf = np.ascontiguousarray(xf).astype(np.float16)
    # weights [p, t, o] fp16: row (t*128 + p) of W -> [p, t, :]
    def wprep(W):
        return np.ascontiguousarray(
            W.reshape(DT, P, H).transpose(1, 0, 2)).astype(np.float16)
    # biases [p, 2, t]
    bqk_h = np.ascontiguousarray(np.stack(
        [bq.reshape(DT, P).T, bk.reshape(DT, P).T], axis=1)).astype(np.float32)

    cos2, sins = _rope_tables()
    cossin = np.ascontiguousarray(np.stack([cos2, sins], axis=1))
    nc = _get_nc()
    in_maps = []
    wq_h, wk_h, wv_h, wo_h = wprep(Wq), wprep(Wk), wprep(Wv), wprep(Wo)
    for c in range(NCORES):
        in_maps.append(dict(
            xsT=np.ascontiguousarray(xf[c * BR:(c + 1) * BR]),
            wq=wq_h, wk=wk_h, wv=wv_h, wo=wo_h, bqk=bqk_h,
            cossin=cossin))
    kw = {}
    if trace:
        import os
        td = "/tmp/trn_trace"
        os.makedirs(td, exist_ok=True)
        kw["tmpdir"] = td
    res = run_bass_kernel_spmd(nc, in_maps, core_ids=list(range(NCORES)),
                               trace=trace, **kw)
    outs = np.concatenate([r["out"] for r in res.results], axis=0)
    out = outs.reshape(2, NCORES * BR // 2, S, H)
    out = out + (bv @ Wo + bo)
    return out.astype(np.float32), res


def kernel(**inputs):
    mask = np.asarray(inputs["mask"])
    if not np.all(mask != 0):
        return _numpy_reference(
            x=np.asarray(inputs["x"], np.float32),
            Wq=np.asarray(inputs["Wq"], np.float32),
            bq=np.asarray(inputs["bq"], np.float32),
            Wk=np.asarray(inputs["Wk"], np.float32),
            bk=np.asarray(inputs["bk"], np.float32),
            Wv=np.asarray(inputs["Wv"], np.float32),
            bv=np.asarray(inputs["bv"], np.float32),
            Wo=np.asarray(inputs["Wo"], np.float32),
            bo=np.asarray(inputs["bo"], np.float32),
            mask=mask)
    out, _ = _run(inputs, trace=False)
    return out
